# revision 9
# baseline (speedup 1.0000x reference)
"""Trainium2 Bass kernel v2 for nn_MCA_12214886990440 (strip-conv dual-axis attention).

Sharding: data-parallel over batch B=8 across 8 NeuronCores (params replicated).

All layout pivots are on-chip xbar DMA transposes (fp16), no DRAM parking.
BN is folded into conv weights host-side. Conv runs col-tiled (2 pixel chunks
concurrently on PE column halves); qkv runs row-tiled (2 chunks on the two PE
row halves). Attention is reassociated through 128x128 Grams:
    G_w[w2,w]   = sum_{d,h} hq[d,h,w2] wv[d,h,w]       (scale folded into Wq)
    w_o[w,(d,h)] = sum_{w2} G_w[w2,w] wk[d,h,w2]
and symmetrically for the h-branch. Final: y = x * sigmoid(wout@w_o + hout@h_o).
"""
import sys
sys.path.insert(0, "/opt/trn_rl_repo")

import numpy as np

import concourse.bass as bass
import concourse.tile as tile
from concourse import bacc
from concourse import mybir

B, C, H, W, NH, D = 8, 64, 128, 128, 8, 8
KS = [7, 11, 21]
EPS = 1e-5
PAD = 10
NTAP = 21
HW = H * W
PADROWS = H + 2 * PAD  # 148
F32 = mybir.dt.float32
F16 = mybir.dt.float16
AF = mybir.ActivationFunctionType
ALU = mybir.AluOpType

N_CORES = 8
CH = 512          # pixel chunk
NPAIR = 16        # chunk pairs (ci, ci+16)


DEBUG = False


def _kernel_body(tc, a):
    nc = tc.nc

    # ---------------- pools (alloc order = reverse release order) -----------
    wp = tc.alloc_tile_pool(name="wts", bufs=1)
    zp = tc.alloc_tile_pool(name="z", bufs=1)
    gp = tc.alloc_tile_pool(name="g", bufs=1)
    scp = tc.alloc_tile_pool(name="sc", bufs=1)
    chp = tc.alloc_tile_pool(name="chan", bufs=1)
    pp = tc.alloc_tile_pool(name="pad", bufs=1)

    # weights
    wconv = wp.tile([128, 2 * 704], F16, tag="wconv", name="wconv")
    nc.sync.dma_start(wconv[:], a["wconv"])
    wqkv = wp.tile([128, 448], F16, tag="wqkv", name="wqkv")
    nc.sync.dma_start(wqkv[:], a["wqkv"])
    wbias = wp.tile([128, 9], F32, tag="wbias", name="wbias")
    nc.sync.dma_start(wbias[:], a["wbias"])
    convw = [wconv[:, 0:704], wconv[:, 704:1408]]
    qkv1w = [wqkv[:, 0:128], wqkv[:, 128:256]]
    qkv2w = [wqkv[:, 256:320], wqkv[:, 320:384]]
    projw = wqkv[:, 384:448]
    convb = [wbias[:, 0:1], wbias[:, 1:2]]
    qkv1b = [wbias[:, 2:3], wbias[:, 3:4]]
    qkv2b = [wbias[:, 4:5], wbias[:, 5:6]]
    projb = wbias[:, 6:7]
    nbv = [wbias[:, 7:8], wbias[:, 8:9]]

    # persistent pivoted tensors (fp16)
    zqv = [zp.tile([128, 128, 128], F16, tag=f"zqv{br}", name=f"zqv{br}")
           for br in range(2)]                       # [h, (w, c)]: c 0-63 q, 64-127 v
    zk = [zp.tile([128, 128, 64], F16, tag=f"zk{br}", name=f"zk{br}")
          for br in range(2)]                        # [w, (h, c)]
    gsb = gp.tile([128, 16 * 128], F16, tag="gsb", name="gsb")

    # ---------------- phase A: conv + qkv per branch ----------------
    ps_conv = tc.alloc_tile_pool(name="ps_conv", bufs=2, space="PSUM")
    ps_qkv1 = tc.alloc_tile_pool(name="ps_qkv1", bufs=1, space="PSUM")
    ps_qkv2 = tc.alloc_tile_pool(name="ps_qkv2", bufs=1, space="PSUM")

    for br in range(2):  # 0 = h-branch (conv along H), 1 = w-branch
        sc = scp.tile([128, 16 * CH], F16, tag="sc", name=f"sc{br}")
        cqv = chp.tile([128, HW], F16, tag="cqv", name=f"cqv{br}")
        ck = chp.tile([128, 8192], F16, tag="ck", name=f"ck{br}")

        pad = pp.tile([128, PADROWS * W], F16, tag="pad", name=f"pad{br}")
        prr = pad[:].rearrange("p (h j) -> p h j", j=PADROWS)
        if br == 0:
            # h-major rows; parts 64-127 = copy shifted one h-row earlier.
            # Borders hold -b/a per channel so folded BN gives 0 there.
            nc.gpsimd.memset(pad[0:64, 0:PAD * W], 0.0)
            nc.gpsimd.memset(pad[0:64, (H + PAD) * W:], 0.0)
            nc.gpsimd.memset(pad[64:128, 0:(PAD - 1) * W], 0.0)
            nc.gpsimd.memset(pad[64:128, (H + PAD - 1) * W:], 0.0)
            nc.vector.tensor_scalar_add(pad[0:64, 0:PAD * W],
                                        pad[0:64, 0:PAD * W], nbv[br][0:64, :])
            nc.vector.tensor_scalar_add(pad[0:64, (H + PAD) * W:],
                                        pad[0:64, (H + PAD) * W:],
                                        nbv[br][0:64, :])
            nc.vector.tensor_scalar_add(pad[64:128, 0:(PAD - 1) * W],
                                        pad[64:128, 0:(PAD - 1) * W],
                                        nbv[br][64:128, :])
            nc.vector.tensor_scalar_add(pad[64:128, (H + PAD - 1) * W:],
                                        pad[64:128, (H + PAD - 1) * W:],
                                        nbv[br][64:128, :])
            for j in range(4):
                r0, r1 = 32 * j, 32 * (j + 1)
                nc.scalar.dma_start(
                    pad[0:64, (PAD + r0) * W:(PAD + r1) * W],
                    a["x16"][:, r0 * W:r1 * W])
                nc.scalar.dma_start(
                    pad[64:128, (PAD - 1 + r0) * W:(PAD - 1 + r1) * W],
                    a["x16"][:, r0 * W:r1 * W])
        else:
            # rows of length 148 (w-padded); parts 64-127 = 1-col shift
            nc.gpsimd.memset(prr[0:64, :, 0:PAD], 0.0)
            nc.gpsimd.memset(prr[0:64, :, H + PAD:], 0.0)
            nc.gpsimd.memset(prr[64:128, :, 0:PAD - 1], 0.0)
            nc.gpsimd.memset(prr[64:128, :, H + PAD - 1:], 0.0)
            nc.vector.tensor_scalar_add(prr[0:64, :, 0:PAD],
                                        prr[0:64, :, 0:PAD], nbv[br][0:64, :])
            nc.vector.tensor_scalar_add(prr[0:64, :, H + PAD:],
                                        prr[0:64, :, H + PAD:],
                                        nbv[br][0:64, :])
            nc.vector.tensor_scalar_add(prr[64:128, :, 0:PAD - 1],
                                        prr[64:128, :, 0:PAD - 1],
                                        nbv[br][64:128, :])
            nc.vector.tensor_scalar_add(prr[64:128, :, H + PAD - 1:],
                                        prr[64:128, :, H + PAD - 1:],
                                        nbv[br][64:128, :])
            xr = a["x16"].rearrange("c (h w) -> c h w", w=W)
            for j in range(4):
                r0, r1 = 32 * j, 32 * (j + 1)
                nc.scalar.dma_start(prr[0:64, r0:r1, PAD:PAD + W],
                                    xr[:, r0:r1, :])
                nc.scalar.dma_start(prr[64:128, r0:r1, PAD - 1:PAD - 1 + W],
                                    xr[:, r0:r1, :])

        # conv: chunk pairs (ci, ci+16) col-tiled on PE column halves.
        # psum parts 0-63 = chunk ci out-chans, parts 64-127 = chunk ci+16.
        cw = convw[br]
        for ci in range(NPAIR):
            psA = ps_conv.tile([128, CH], F32, tag="conv_a")
            psB = ps_conv.tile([128, CH], F32, tag="conv_b")
            for g in range(10):
                w_g = cw[:, g * 64:(g + 1) * 64]
                if br == 0:
                    rA = pad[:, (4 * ci + 2 * g) * W:(4 * ci + 2 * g) * W + CH]
                    rB = pad[:, (4 * ci + 64 + 2 * g) * W:
                             (4 * ci + 64 + 2 * g) * W + CH]
                else:
                    rA = prr[:, 4 * ci:4 * ci + 4, 2 * g:2 * g + W]
                    rB = prr[:, 4 * ci + 64:4 * ci + 68, 2 * g:2 * g + W]
                nc.tensor.matmul(psA[0:64, :], w_g, rA,
                                 start=(g == 0), stop=False)
                nc.tensor.matmul(psB[64:128, :], w_g, rB,
                                 start=(g == 0), stop=False)
            w_g = cw[0:64, 640:704]
            if br == 0:
                rA = pad[0:64, (4 * ci + 20) * W:(4 * ci + 20) * W + CH]
                rB = pad[0:64, (4 * ci + 84) * W:(4 * ci + 84) * W + CH]
            else:
                rA = prr[0:64, 4 * ci:4 * ci + 4, 20:20 + W]
                rB = prr[0:64, 4 * ci + 64:4 * ci + 68, 20:20 + W]
            nc.tensor.matmul(psA[0:64, :], w_g, rA, start=False, stop=True)
            nc.tensor.matmul(psB[64:128, :], w_g, rB, start=False, stop=True)
            nc.scalar.activation(sc[0:64, ci * CH:(ci + 1) * CH], psA[0:64, :],
                                 AF.Identity, bias=convb[br][0:64, :])
            nc.scalar.activation(sc[64:128, ci * CH:(ci + 1) * CH],
                                 psB[64:128, :], AF.Identity,
                                 bias=convb[br][64:128, :])

        # qkv1 (q|v, M=128): w-major pixel streams so the xbar transpose
        # lands h on partitions. Row-tiled K=64 x2 over the two h-halves.
        # cqv layout: [c, (w, h)], h inner 128.
        scrA = sc[0:64, :].rearrange("c (q h w) -> c w (q h)", h=4, w=W)
        scrB = sc[64:128, :].rearrange("c (q h w) -> c w (q h)", h=4, w=W)
        cqr = cqv[:].rearrange("c (w h) -> c w h", h=H)
        for wi in range(16):  # 8 w-columns -> N=512
            psA = ps_qkv1.tile([128, CH], F32, tag="qkv1a")
            psB = ps_qkv1.tile([128, CH], F32, tag="qkv1b")
            nc.tensor.matmul(psA[:], qkv1w[br][0:64, :],
                             scrA[:, 8 * wi:8 * wi + 8, :],
                             start=True, stop=True)
            nc.tensor.matmul(psB[:], qkv1w[br][64:128, :],
                             scrB[:, 8 * wi:8 * wi + 8, :],
                             start=True, stop=True)
            nc.scalar.activation(cqr[:, 8 * wi:8 * wi + 8, 0:64], psA[:],
                                 AF.Identity, bias=qkv1b[br])
            nc.scalar.activation(cqr[:, 8 * wi:8 * wi + 8, 64:128], psB[:],
                                 AF.Identity, bias=qkv1b[br])
        nc.sync.dma_start_transpose(zqv[br][:], cqv[:])

        # qkv2 (k, M=64): h-major pixel chunks (w inner 128) so the xbar
        # transpose lands w on partitions. ck parts 0-63: [c, (h 0-63, w)],
        # parts 64-127: [c, (h 64-127, w)].
        for ci in range(NPAIR):
            ps2a = ps_qkv2.tile([128, CH], F32, tag="qkv2a")
            ps2b = ps_qkv2.tile([128, CH], F32, tag="qkv2b")
            nc.tensor.matmul(ps2a[0:64, :], qkv2w[br][0:64, :],
                             sc[0:64, ci * CH:(ci + 1) * CH],
                             start=True, stop=True)
            nc.tensor.matmul(ps2b[64:128, :], qkv2w[br][64:128, :],
                             sc[64:128, ci * CH:(ci + 1) * CH],
                             start=True, stop=True)
            nc.vector.tensor_scalar_add(ck[0:64, ci * CH:(ci + 1) * CH],
                                        ps2a[0:64, :], qkv2b[br][0:64, :])
            nc.vector.tensor_scalar_add(ck[64:128, ci * CH:(ci + 1) * CH],
                                        ps2b[64:128, :], qkv2b[br][64:128, :])
        nc.sync.dma_start_transpose(zk[br][:, 0:64, :], ck[0:64, :])
        nc.sync.dma_start_transpose(zk[br][:, 64:128, :], ck[64:128, :])

        if DEBUG:
            nc.sync.dma_start(a[f"dbg_sc{br}"], sc[:])
            nc.sync.dma_start(a[f"dbg_zqv{br}"],
                              zqv[br][:].rearrange("h w c -> h (w c)"))
            nc.sync.dma_start(a[f"dbg_zk{br}"],
                              zk[br][:].rearrange("w h c -> w (h c)"))

    ps_qkv2.release()
    ps_qkv1.release()
    ps_conv.release()
    pp.release()
    chp.release()
    scp.release()

    # ---------------- phase B: attention ----------------
    zsp = tc.alloc_tile_pool(name="zs", bufs=1)
    sp = tc.alloc_tile_pool(name="s", bufs=1)
    rp = tc.alloc_tile_pool(name="ring", bufs=2)
    ps_g = tc.alloc_tile_pool(name="ps_g", bufs=2, space="PSUM")
    ps_bt = tc.alloc_tile_pool(name="ps_bt", bufs=2, space="PSUM")
    ps_pj = tc.alloc_tile_pool(name="ps_pj", bufs=2, space="PSUM")

    zs = zsp.tile([128, 16384], F16, tag="zs", name="zs")   # [w, (h, c)]
    zsr = zs[:].rearrange("w (h c) -> w c h", c=128)
    s_cp = sp.tile([128, 128, 128], F16, tag="scp", name="scp")  # [c, h, w]

    # Grams: gi=0: G_w = sum_d hq^T wv; gi=1: G_h = sum_d wq^T hv
    for gi in range(2):
        zq = zqv[0] if gi == 0 else zqv[1]
        zv = zqv[1] if gi == 0 else zqv[0]
        for n in range(NH):
            gps = ps_g.tile([128, CH], F32, tag="g")
            for d in range(D):
                c = n * D + d
                lhs = zq[:, :, c:c + 1].rearrange("h w e -> h (w e)")
                rhs = zv[:, :, 64 + c:65 + c].rearrange("h w e -> h (w e)")
                nc.tensor.matmul(gps[:, 0:128], lhs, rhs,
                                 start=(d == 0), stop=(d == D - 1))
            nc.scalar.activation(
                gsb[:, (gi * NH + n) * 128:(gi * NH + n + 1) * 128],
                gps[:, 0:128], AF.Copy)

    # B^T: o[w, (h, d)] = sum_{w2} G[w2, w] * k[d, h, w2]
    for gi in range(2):
        zkk = zk[1] if gi == 0 else zk[0]   # w_o uses wk; h_o uses hk
        for n in range(NH):
            g_ap = gsb[:, (gi * NH + n) * 128:(gi * NH + n + 1) * 128]
            for j in range(2):
                bps = ps_bt.tile([128, CH], F32, tag="bt")
                rhs = zkk[:, :, n * D + 4 * j:n * D + 4 * j + 4].rearrange(
                    "w h d -> w d h")
                nc.tensor.matmul(bps[:], g_ap, rhs, start=True, stop=True)
                c0 = gi * 64 + n * D + 4 * j
                nc.scalar.activation(zsr[:, c0:c0 + 4, :], bps[:], AF.Copy)

    if DEBUG:
        nc.sync.dma_start(a["dbg_gsb"], gsb[:])
        nc.sync.dma_start(a["dbg_zs"], zs[:])

    # S pivot: [w, (h, c)] -> [c, h, w], 4 h-quarter transposes
    for q in range(4):
        nc.sync.dma_start_transpose(
            s_cp[:, q * 32:(q + 1) * 32, :], zs[:, q * 4096:(q + 1) * 4096])

    if DEBUG:
        nc.sync.dma_start(a["dbg_scp"], s_cp[:].rearrange("c h w -> c (h w)"))

    # projection (col-tiled pairs) + sigmoid + x*sig -> y
    s_flat = s_cp[:].rearrange("c a b -> c (a b)")
    for ci in range(NPAIR):
        ppsA = ps_pj.tile([128, CH], F32, tag="pj_a")
        ppsB = ps_pj.tile([128, CH], F32, tag="pj_b")
        nc.tensor.matmul(ppsA[0:64, :], projw,
                         s_flat[:, ci * CH:(ci + 1) * CH],
                         start=True, stop=True)
        nc.tensor.matmul(ppsB[64:128, :], projw,
                         s_flat[:, (ci + 16) * CH:(ci + 17) * CH],
                         start=True, stop=True)
        sg = rp.tile([128, CH], F32, tag="sg")
        nc.scalar.activation(sg[0:64, :], ppsA[0:64, :], AF.Sigmoid,
                             bias=projb[0:64, :])
        nc.scalar.activation(sg[64:128, :], ppsB[64:128, :], AF.Sigmoid,
                             bias=projb[64:128, :])
        xc = rp.tile([128, CH], F32, tag="xc")
        nc.scalar.dma_start(xc[0:64, :], a["x"][:, ci * CH:(ci + 1) * CH])
        nc.scalar.dma_start(xc[64:128, :],
                            a["x"][:, (ci + 16) * CH:(ci + 17) * CH])
        yt = rp.tile([128, CH], F32, tag="yt")
        nc.vector.tensor_mul(yt[:], sg[:], xc[:])
        nc.sync.dma_start(a["y"][:, ci * CH:(ci + 1) * CH], yt[0:64, :])
        nc.sync.dma_start(a["y"][:, (ci + 16) * CH:(ci + 17) * CH],
                          yt[64:128, :])

    for p in (ps_pj, ps_bt, ps_g, rp, sp, zsp, gp, zp, wp):
        p.release()


def _prep_weights(inputs):
    """Host-side packing: BN folded into conv weights, qkv biases folded."""
    inp = {k: np.asarray(v, dtype=np.float64) for k, v in inputs.items()}
    w = {}
    a1 = inp["bn1_g"] / np.sqrt(inp["bn1_v"] + EPS)
    b1 = inp["bn1_b"] - inp["bn1_m"] * a1
    a2 = inp["bn2_g"] / np.sqrt(inp["bn2_v"] + EPS)
    b2 = inp["bn2_b"] - inp["bn2_m"] * a2

    def conv_pack(ws, ab, bb, bias):
        # eff[t][o, i]; BN: x_bn = a*x + b folded: W' = W*diag(a), b' += sum_t W_t@b
        eff = np.zeros((NTAP, C, C))
        for j, k in enumerate(KS):
            off = PAD - k // 2
            for i in range(k):
                eff[off + i] += ws[j][:, :, i]
        bconv = bias + sum(eff[t] @ bb for t in range(NTAP))
        effs = eff * ab[None, None, :]
        pk = np.zeros((128, 704))
        for g in range(10):
            pk[0:64, g * 64:(g + 1) * 64] = effs[2 * g].T
            pk[64:128, g * 64:(g + 1) * 64] = effs[2 * g + 1].T
        pk[0:64, 640:704] = effs[20].T
        return pk, bconv

    pk_h, bc_h = conv_pack([inp[f"sc1_w{j}"][:, :, :, 0] for j in range(3)],
                           a1, b1, inp["sc1_b0"] + inp["sc1_b1"] + inp["sc1_b2"])
    pk_w, bc_w = conv_pack([inp[f"sc2_w{j}"][:, :, 0, :] for j in range(3)],
                           a2, b2, inp["sc2_b0"] + inp["sc2_b1"] + inp["sc2_b2"])

    scale = D * H ** (-0.5)
    idx = (np.arange(NH)[:, None] * 24 + np.arange(D)[None, :]).ravel()
    idx_q, idx_k, idx_v = idx, idx + 8, idx + 16

    wqkv = np.zeros((128, 448))
    wbias = np.zeros((128, 9))
    wbias[:, 0] = np.tile(bc_h, 2)
    wbias[:, 1] = np.tile(bc_w, 2)
    for br, (qw, qb, bc) in enumerate(
            [(inp["hqkv_w"], inp["hqkv_b"], bc_h),
             (inp["wqkv_w"], inp["wqkv_b"], bc_w)]):
        bfold = qb
        Wq, Wk, Wv = qw[idx_q] * scale, qw[idx_k], qw[idx_v]
        bq, bk, bv = bfold[idx_q] * scale, bfold[idx_k], bfold[idx_v]
        q1 = np.concatenate([Wq.T, Wv.T], axis=1)          # [64, 128]
        wqkv[:, br * 128:(br + 1) * 128] = np.tile(q1, (2, 1))
        wqkv[:, 256 + br * 64:256 + (br + 1) * 64] = np.tile(Wk.T, (2, 1))
        wbias[:, 2 + br] = np.concatenate([bq, bv])
        wbias[:, 4 + br] = np.tile(bk, 2)
    wqkv[:, 384:448] = np.concatenate([inp["wout_w"].T, inp["hout_w"].T],
                                      axis=0)              # [128, 64]
    wbias[:, 6] = np.tile(inp["wout_b"] + inp["hout_b"], 2)
    wbias[:, 7] = np.tile(-b1 / a1, 2)
    wbias[:, 8] = np.tile(-b2 / a2, 2)

    wconv = np.concatenate([pk_h, pk_w], axis=1)           # [128, 1408]
    return {"wconv": wconv.astype(np.float16),
            "wqkv": wqkv.astype(np.float16),
            "wbias": wbias.astype(np.float32)}


_NC_CACHE = {}
_RUN_OPTS = {"trace": False}
_LAST_RESULT = {}

_SHAPES = {"x": ([C, HW], F32), "x16": ([C, HW], F16),
           "wconv": ([128, 1408], F16), "wqkv": ([128, 448], F16),
           "wbias": ([128, 9], F32)}


def _build_nc():
    if "nc" in _NC_CACHE:
        return _NC_CACHE["nc"]
    nc = bacc.Bacc(trn_type="TRN2", target_bir_lowering=False, debug=False)
    a = {}
    for n, (s, dt) in _SHAPES.items():
        a[n] = nc.dram_tensor(n, s, dt, kind="ExternalInput").ap()
    a["y"] = nc.dram_tensor("y", [C, HW], F32, kind="ExternalOutput").ap()
    if _kernel_body.__globals__["DEBUG"]:
        dbg = {"dbg_sc0": [128, 8192], "dbg_sc1": [128, 8192],
               "dbg_zqv0": [128, HW], "dbg_zqv1": [128, HW],
               "dbg_zk0": [128, 8192], "dbg_zk1": [128, 8192],
               "dbg_gsb": [128, 2048], "dbg_zs": [128, HW],
               "dbg_scp": [128, HW]}
        for n, s in dbg.items():
            a[n] = nc.dram_tensor(n, s, F16, kind="ExternalOutput").ap()
    with tile.TileContext(nc) as tc:
        _kernel_body(tc, a)
    nc.compile()
    _NC_CACHE["nc"] = nc
    return nc


def _in_maps(inputs):
    w = _prep_weights(inputs)
    x = np.ascontiguousarray(np.asarray(inputs["x"], dtype=np.float32))
    maps = []
    for core in range(N_CORES):
        xc = np.ascontiguousarray(x[core].reshape(C, HW))
        m = {"x": xc, "x16": xc.astype(np.float16)}
        m.update(w)
        maps.append(m)
    return maps


def kernel(**inputs):
    from concourse.bass_utils import run_bass_kernel_spmd

    nc = _build_nc()
    res = run_bass_kernel_spmd(nc, _in_maps(inputs), core_ids=list(range(N_CORES)),
                               trace=_RUN_OPTS["trace"])
    _LAST_RESULT["res"] = res
    out = np.stack([res.results[i]["y"].reshape(C, H, W) for i in range(N_CORES)])
    return out.astype(np.float32)


if __name__ == "__main__":
    nc = _build_nc()
    print("built ok")


# revision 10
# speedup vs baseline: 1.0165x; 1.0165x over previous
"""Trainium2 Bass kernel v2 for nn_MCA_12214886990440 (strip-conv dual-axis attention).

Sharding: data-parallel over batch B=8 across 8 NeuronCores (params replicated).

All layout pivots are on-chip xbar DMA transposes (fp16), no DRAM parking.
BN is folded into conv weights host-side. Conv runs col-tiled (2 pixel chunks
concurrently on PE column halves); qkv runs row-tiled (2 chunks on the two PE
row halves). Attention is reassociated through 128x128 Grams:
    G_w[w2,w]   = sum_{d,h} hq[d,h,w2] wv[d,h,w]       (scale folded into Wq)
    w_o[w,(d,h)] = sum_{w2} G_w[w2,w] wk[d,h,w2]
and symmetrically for the h-branch. Final: y = x * sigmoid(wout@w_o + hout@h_o).
"""
import sys
sys.path.insert(0, "/opt/trn_rl_repo")

import numpy as np

import concourse.bass as bass
import concourse.tile as tile
from concourse import bacc
from concourse import mybir

B, C, H, W, NH, D = 8, 64, 128, 128, 8, 8
KS = [7, 11, 21]
EPS = 1e-5
PAD = 10
NTAP = 21
HW = H * W
PADROWS = H + 2 * PAD  # 148
F32 = mybir.dt.float32
F16 = mybir.dt.float16
AF = mybir.ActivationFunctionType
ALU = mybir.AluOpType

N_CORES = 8
CH = 512          # pixel chunk
NPAIR = 16        # chunk pairs (ci, ci+16)


DEBUG = False


def _kernel_body(tc, a):
    nc = tc.nc

    # ---------------- pools (alloc order = reverse release order) -----------
    wp = tc.alloc_tile_pool(name="wts", bufs=1)
    zp = tc.alloc_tile_pool(name="z", bufs=1)
    gp = tc.alloc_tile_pool(name="g", bufs=1)
    scp = tc.alloc_tile_pool(name="sc", bufs=1)
    chp = tc.alloc_tile_pool(name="chan", bufs=1)
    pp = tc.alloc_tile_pool(name="pad", bufs=1)

    # weights
    wconv = wp.tile([128, 2 * 704], F16, tag="wconv", name="wconv")
    nc.sync.dma_start(wconv[:], a["wconv"])
    wqkv = wp.tile([128, 448], F16, tag="wqkv", name="wqkv")
    nc.sync.dma_start(wqkv[:], a["wqkv"])
    wbias = wp.tile([128, 9], F32, tag="wbias", name="wbias")
    nc.sync.dma_start(wbias[:], a["wbias"])
    convw = [wconv[:, 0:704], wconv[:, 704:1408]]
    qkv1w = [wqkv[:, 0:128], wqkv[:, 128:256]]
    qkv2w = [wqkv[:, 256:320], wqkv[:, 320:384]]
    projw = wqkv[:, 384:448]
    convb = [wbias[:, 0:1], wbias[:, 1:2]]
    qkv1b = [wbias[:, 2:3], wbias[:, 3:4]]
    qkv2b = [wbias[:, 4:5], wbias[:, 5:6]]
    projb = wbias[:, 6:7]
    nbv = [wbias[:, 7:8], wbias[:, 8:9]]

    # persistent pivoted tensors (fp16)
    zqv = [zp.tile([128, 128, 128], F16, tag=f"zqv{br}", name=f"zqv{br}")
           for br in range(2)]                       # [h, (w, c)]: c 0-63 q, 64-127 v
    zk = [zp.tile([128, 128, 64], F16, tag=f"zk{br}", name=f"zk{br}")
          for br in range(2)]                        # [w, (h, c)]
    gsb = gp.tile([128, 16 * 128], F16, tag="gsb", name="gsb")

    # ---------------- phase A: conv + qkv per branch ----------------
    ps_conv = tc.alloc_tile_pool(name="ps_conv", bufs=2, space="PSUM")
    ps_qkv1 = tc.alloc_tile_pool(name="ps_qkv1", bufs=1, space="PSUM")
    ps_qkv2 = tc.alloc_tile_pool(name="ps_qkv2", bufs=1, space="PSUM")

    for br in range(2):  # 0 = h-branch (conv along H), 1 = w-branch
        sc = scp.tile([128, 16 * CH], F16, tag="sc", name=f"sc{br}")
        cqv = chp.tile([128, HW], F16, tag="cqv", name=f"cqv{br}")
        ck = chp.tile([128, 8192], F16, tag="ck", name=f"ck{br}")

        pad = pp.tile([128, PADROWS * W], F16, tag="pad", name=f"pad{br}")
        prr = pad[:].rearrange("p (h j) -> p h j", j=PADROWS)
        if br == 0:
            # h-major rows; parts 64-127 = copy shifted one h-row earlier.
            # Borders hold -b/a per channel so folded BN gives 0 there.
            nc.gpsimd.memset(pad[0:64, 0:PAD * W], 0.0)
            nc.gpsimd.memset(pad[0:64, (H + PAD) * W:], 0.0)
            nc.gpsimd.memset(pad[64:128, 0:(PAD - 1) * W], 0.0)
            nc.gpsimd.memset(pad[64:128, (H + PAD - 1) * W:], 0.0)
            nc.vector.tensor_scalar_add(pad[0:64, 0:PAD * W],
                                        pad[0:64, 0:PAD * W], nbv[br][0:64, :])
            nc.vector.tensor_scalar_add(pad[0:64, (H + PAD) * W:],
                                        pad[0:64, (H + PAD) * W:],
                                        nbv[br][0:64, :])
            nc.vector.tensor_scalar_add(pad[64:128, 0:(PAD - 1) * W],
                                        pad[64:128, 0:(PAD - 1) * W],
                                        nbv[br][64:128, :])
            nc.vector.tensor_scalar_add(pad[64:128, (H + PAD - 1) * W:],
                                        pad[64:128, (H + PAD - 1) * W:],
                                        nbv[br][64:128, :])
            for j in range(4):
                r0, r1 = 32 * j, 32 * (j + 1)
                nc.scalar.dma_start(
                    pad[0:64, (PAD + r0) * W:(PAD + r1) * W],
                    a["x16"][:, r0 * W:r1 * W])
                nc.scalar.dma_start(
                    pad[64:128, (PAD - 1 + r0) * W:(PAD - 1 + r1) * W],
                    a["x16"][:, r0 * W:r1 * W])
        else:
            # rows of length 148 (w-padded); parts 64-127 = 1-col shift
            nc.gpsimd.memset(prr[0:64, :, 0:PAD], 0.0)
            nc.gpsimd.memset(prr[0:64, :, H + PAD:], 0.0)
            nc.gpsimd.memset(prr[64:128, :, 0:PAD - 1], 0.0)
            nc.gpsimd.memset(prr[64:128, :, H + PAD - 1:], 0.0)
            nc.vector.tensor_scalar_add(prr[0:64, :, 0:PAD],
                                        prr[0:64, :, 0:PAD], nbv[br][0:64, :])
            nc.vector.tensor_scalar_add(prr[0:64, :, H + PAD:],
                                        prr[0:64, :, H + PAD:],
                                        nbv[br][0:64, :])
            nc.vector.tensor_scalar_add(prr[64:128, :, 0:PAD - 1],
                                        prr[64:128, :, 0:PAD - 1],
                                        nbv[br][64:128, :])
            nc.vector.tensor_scalar_add(prr[64:128, :, H + PAD - 1:],
                                        prr[64:128, :, H + PAD - 1:],
                                        nbv[br][64:128, :])
            xr = a["x16"].rearrange("c (h w) -> c h w", w=W)
            for j in range(4):
                r0, r1 = 32 * j, 32 * (j + 1)
                nc.scalar.dma_start(prr[0:64, r0:r1, PAD:PAD + W],
                                    xr[:, r0:r1, :])
                nc.scalar.dma_start(prr[64:128, r0:r1, PAD - 1:PAD - 1 + W],
                                    xr[:, r0:r1, :])

        # conv: chunk pairs (ci, ci+16) col-tiled on PE column halves.
        # psum parts 0-63 = chunk ci out-chans, parts 64-127 = chunk ci+16.
        cw = convw[br]
        for ci in range(NPAIR):
            psA = ps_conv.tile([128, CH], F32, tag="conv_a")
            psB = ps_conv.tile([128, CH], F32, tag="conv_b")
            for g in range(10):
                w_g = cw[:, g * 64:(g + 1) * 64]
                if br == 0:
                    rA = pad[:, (4 * ci + 2 * g) * W:(4 * ci + 2 * g) * W + CH]
                    rB = pad[:, (4 * ci + 64 + 2 * g) * W:
                             (4 * ci + 64 + 2 * g) * W + CH]
                else:
                    rA = prr[:, 4 * ci:4 * ci + 4, 2 * g:2 * g + W]
                    rB = prr[:, 4 * ci + 64:4 * ci + 68, 2 * g:2 * g + W]
                nc.tensor.matmul(psA[0:64, :], w_g, rA,
                                 start=(g == 0), stop=False)
                nc.tensor.matmul(psB[64:128, :], w_g, rB,
                                 start=(g == 0), stop=False)
            w_g = cw[0:64, 640:704]
            if br == 0:
                rA = pad[0:64, (4 * ci + 20) * W:(4 * ci + 20) * W + CH]
                rB = pad[0:64, (4 * ci + 84) * W:(4 * ci + 84) * W + CH]
            else:
                rA = prr[0:64, 4 * ci:4 * ci + 4, 20:20 + W]
                rB = prr[0:64, 4 * ci + 64:4 * ci + 68, 20:20 + W]
            nc.tensor.matmul(psA[0:64, :], w_g, rA, start=False, stop=True)
            nc.tensor.matmul(psB[64:128, :], w_g, rB, start=False, stop=True)
            nc.scalar.activation(sc[0:64, ci * CH:(ci + 1) * CH], psA[0:64, :],
                                 AF.Identity, bias=convb[br][0:64, :])
            nc.scalar.activation(sc[64:128, ci * CH:(ci + 1) * CH],
                                 psB[64:128, :], AF.Identity,
                                 bias=convb[br][64:128, :])

        # qkv1 (q|v, M=128): w-major pixel streams so the xbar transpose
        # lands h on partitions. Row-tiled K=64 x2 over the two h-halves.
        # cqv layout: [c, (w, h)], h inner 128.
        scrA = sc[0:64, :].rearrange("c (q h w) -> c w (q h)", h=4, w=W)
        scrB = sc[64:128, :].rearrange("c (q h w) -> c w (q h)", h=4, w=W)
        cqr = cqv[:].rearrange("c (w h) -> c w h", h=H)
        for wi in range(16):  # 8 w-columns -> N=512
            psA = ps_qkv1.tile([128, CH], F32, tag="qkv1a")
            psB = ps_qkv1.tile([128, CH], F32, tag="qkv1b")
            nc.tensor.matmul(psA[:], qkv1w[br][0:64, :],
                             scrA[:, 8 * wi:8 * wi + 8, :],
                             start=True, stop=True)
            nc.tensor.matmul(psB[:], qkv1w[br][64:128, :],
                             scrB[:, 8 * wi:8 * wi + 8, :],
                             start=True, stop=True)
            nc.scalar.activation(cqr[:, 8 * wi:8 * wi + 8, 0:64], psA[:],
                                 AF.Identity, bias=qkv1b[br])
            nc.scalar.activation(cqr[:, 8 * wi:8 * wi + 8, 64:128], psB[:],
                                 AF.Identity, bias=qkv1b[br])
        nc.sync.dma_start_transpose(zqv[br][:], cqv[:])

        # qkv2 (k, M=64): h-major pixel chunks (w inner 128) so the xbar
        # transpose lands w on partitions. ck parts 0-63: [c, (h 0-63, w)],
        # parts 64-127: [c, (h 64-127, w)].
        for ci in range(NPAIR):
            ps2a = ps_qkv2.tile([128, CH], F32, tag="qkv2a")
            ps2b = ps_qkv2.tile([128, CH], F32, tag="qkv2b")
            nc.tensor.matmul(ps2a[0:64, :], qkv2w[br][0:64, :],
                             sc[0:64, ci * CH:(ci + 1) * CH],
                             start=True, stop=True)
            nc.tensor.matmul(ps2b[64:128, :], qkv2w[br][64:128, :],
                             sc[64:128, ci * CH:(ci + 1) * CH],
                             start=True, stop=True)
            nc.vector.tensor_scalar_add(ck[0:64, ci * CH:(ci + 1) * CH],
                                        ps2a[0:64, :], qkv2b[br][0:64, :])
            nc.vector.tensor_scalar_add(ck[64:128, ci * CH:(ci + 1) * CH],
                                        ps2b[64:128, :], qkv2b[br][64:128, :])
        nc.sync.dma_start_transpose(zk[br][:, 0:64, :], ck[0:64, :])
        nc.sync.dma_start_transpose(zk[br][:, 64:128, :], ck[64:128, :])

        if DEBUG:
            nc.sync.dma_start(a[f"dbg_sc{br}"], sc[:])
            nc.sync.dma_start(a[f"dbg_zqv{br}"],
                              zqv[br][:].rearrange("h w c -> h (w c)"))
            nc.sync.dma_start(a[f"dbg_zk{br}"],
                              zk[br][:].rearrange("w h c -> w (h c)"))

    ps_qkv2.release()
    ps_qkv1.release()
    ps_conv.release()
    pp.release()
    chp.release()
    scp.release()

    # ---------------- phase B: attention ----------------
    zsp = tc.alloc_tile_pool(name="zs", bufs=1)
    sp = tc.alloc_tile_pool(name="s", bufs=1)
    rp = tc.alloc_tile_pool(name="ring", bufs=2)
    ps_g = tc.alloc_tile_pool(name="ps_g", bufs=2, space="PSUM")
    ps_bt = tc.alloc_tile_pool(name="ps_bt", bufs=2, space="PSUM")
    ps_pj = tc.alloc_tile_pool(name="ps_pj", bufs=2, space="PSUM")

    zs = zsp.tile([128, 16384], F16, tag="zs", name="zs")   # [w, (h, c)]
    zsr = zs[:].rearrange("w (h c) -> w c h", c=128)
    s_cp = sp.tile([128, 128, 128], F16, tag="scp", name="scp")  # [c, h, w]

    # Gram + B^T fused per head (B^T follows its head's Gram immediately
    # so the S pivot can start right after the last head instead of a full
    # B^T phase later).
    for gi in range(2):
        zq = zqv[0] if gi == 0 else zqv[1]
        zv = zqv[1] if gi == 0 else zqv[0]
        zkk = zk[1] if gi == 0 else zk[0]   # w_o uses wk; h_o uses hk
        for n in range(NH):
            gps = ps_g.tile([128, CH], F32, tag="g")
            for d in range(D):
                c = n * D + d
                lhs = zq[:, :, c:c + 1].rearrange("h w e -> h (w e)")
                rhs = zv[:, :, 64 + c:65 + c].rearrange("h w e -> h (w e)")
                nc.tensor.matmul(gps[:, 0:128], lhs, rhs,
                                 start=(d == 0), stop=(d == D - 1))
            g_ap = gsb[:, (gi * NH + n) * 128:(gi * NH + n + 1) * 128]
            nc.scalar.activation(g_ap, gps[:, 0:128], AF.Copy)
            for j in range(2):
                bps = ps_bt.tile([128, CH], F32, tag="bt")
                rhs = zkk[:, :, n * D + 4 * j:n * D + 4 * j + 4].rearrange(
                    "w h d -> w d h")
                nc.tensor.matmul(bps[:], g_ap, rhs, start=True, stop=True)
                c0 = gi * 64 + n * D + 4 * j
                nc.scalar.activation(zsr[:, c0:c0 + 4, :], bps[:], AF.Copy)

    if DEBUG:
        nc.sync.dma_start(a["dbg_gsb"], gsb[:])
        nc.sync.dma_start(a["dbg_zs"], zs[:])

    # S pivot: [w, (h, c)] -> [c, h, w], 4 h-quarter transposes
    for q in range(4):
        nc.sync.dma_start_transpose(
            s_cp[:, q * 32:(q + 1) * 32, :], zs[:, q * 4096:(q + 1) * 4096])

    if DEBUG:
        nc.sync.dma_start(a["dbg_scp"], s_cp[:].rearrange("c h w -> c (h w)"))

    # projection (col-tiled pairs) + sigmoid + x*sig -> y
    s_flat = s_cp[:].rearrange("c a b -> c (a b)")
    for ci in range(NPAIR):
        ppsA = ps_pj.tile([128, CH], F32, tag="pj_a")
        ppsB = ps_pj.tile([128, CH], F32, tag="pj_b")
        nc.tensor.matmul(ppsA[0:64, :], projw,
                         s_flat[:, ci * CH:(ci + 1) * CH],
                         start=True, stop=True)
        nc.tensor.matmul(ppsB[64:128, :], projw,
                         s_flat[:, (ci + 16) * CH:(ci + 17) * CH],
                         start=True, stop=True)
        sg = rp.tile([128, CH], F32, tag="sg")
        nc.scalar.activation(sg[0:64, :], ppsA[0:64, :], AF.Sigmoid,
                             bias=projb[0:64, :])
        nc.scalar.activation(sg[64:128, :], ppsB[64:128, :], AF.Sigmoid,
                             bias=projb[64:128, :])
        xc = rp.tile([128, CH], F32, tag="xc")
        nc.scalar.dma_start(xc[0:64, :], a["x"][:, ci * CH:(ci + 1) * CH])
        nc.scalar.dma_start(xc[64:128, :],
                            a["x"][:, (ci + 16) * CH:(ci + 17) * CH])
        yt = rp.tile([128, CH], F32, tag="yt")
        nc.vector.tensor_mul(yt[:], sg[:], xc[:])
        nc.sync.dma_start(a["y"][:, ci * CH:(ci + 1) * CH], yt[0:64, :])
        nc.sync.dma_start(a["y"][:, (ci + 16) * CH:(ci + 17) * CH],
                          yt[64:128, :])

    for p in (ps_pj, ps_bt, ps_g, rp, sp, zsp, gp, zp, wp):
        p.release()


def _prep_weights(inputs):
    """Host-side packing: BN folded into conv weights, qkv biases folded."""
    inp = {k: np.asarray(v, dtype=np.float64) for k, v in inputs.items()}
    w = {}
    a1 = inp["bn1_g"] / np.sqrt(inp["bn1_v"] + EPS)
    b1 = inp["bn1_b"] - inp["bn1_m"] * a1
    a2 = inp["bn2_g"] / np.sqrt(inp["bn2_v"] + EPS)
    b2 = inp["bn2_b"] - inp["bn2_m"] * a2

    def conv_pack(ws, ab, bb, bias):
        # eff[t][o, i]; BN: x_bn = a*x + b folded: W' = W*diag(a), b' += sum_t W_t@b
        eff = np.zeros((NTAP, C, C))
        for j, k in enumerate(KS):
            off = PAD - k // 2
            for i in range(k):
                eff[off + i] += ws[j][:, :, i]
        bconv = bias + sum(eff[t] @ bb for t in range(NTAP))
        effs = eff * ab[None, None, :]
        pk = np.zeros((128, 704))
        for g in range(10):
            pk[0:64, g * 64:(g + 1) * 64] = effs[2 * g].T
            pk[64:128, g * 64:(g + 1) * 64] = effs[2 * g + 1].T
        pk[0:64, 640:704] = effs[20].T
        return pk, bconv

    pk_h, bc_h = conv_pack([inp[f"sc1_w{j}"][:, :, :, 0] for j in range(3)],
                           a1, b1, inp["sc1_b0"] + inp["sc1_b1"] + inp["sc1_b2"])
    pk_w, bc_w = conv_pack([inp[f"sc2_w{j}"][:, :, 0, :] for j in range(3)],
                           a2, b2, inp["sc2_b0"] + inp["sc2_b1"] + inp["sc2_b2"])

    scale = D * H ** (-0.5)
    idx = (np.arange(NH)[:, None] * 24 + np.arange(D)[None, :]).ravel()
    idx_q, idx_k, idx_v = idx, idx + 8, idx + 16

    wqkv = np.zeros((128, 448))
    wbias = np.zeros((128, 9))
    wbias[:, 0] = np.tile(bc_h, 2)
    wbias[:, 1] = np.tile(bc_w, 2)
    for br, (qw, qb, bc) in enumerate(
            [(inp["hqkv_w"], inp["hqkv_b"], bc_h),
             (inp["wqkv_w"], inp["wqkv_b"], bc_w)]):
        bfold = qb
        Wq, Wk, Wv = qw[idx_q] * scale, qw[idx_k], qw[idx_v]
        bq, bk, bv = bfold[idx_q] * scale, bfold[idx_k], bfold[idx_v]
        q1 = np.concatenate([Wq.T, Wv.T], axis=1)          # [64, 128]
        wqkv[:, br * 128:(br + 1) * 128] = np.tile(q1, (2, 1))
        wqkv[:, 256 + br * 64:256 + (br + 1) * 64] = np.tile(Wk.T, (2, 1))
        wbias[:, 2 + br] = np.concatenate([bq, bv])
        wbias[:, 4 + br] = np.tile(bk, 2)
    wqkv[:, 384:448] = np.concatenate([inp["wout_w"].T, inp["hout_w"].T],
                                      axis=0)              # [128, 64]
    wbias[:, 6] = np.tile(inp["wout_b"] + inp["hout_b"], 2)
    wbias[:, 7] = np.tile(-b1 / a1, 2)
    wbias[:, 8] = np.tile(-b2 / a2, 2)

    wconv = np.concatenate([pk_h, pk_w], axis=1)           # [128, 1408]
    return {"wconv": wconv.astype(np.float16),
            "wqkv": wqkv.astype(np.float16),
            "wbias": wbias.astype(np.float32)}


_NC_CACHE = {}
_RUN_OPTS = {"trace": False}
_LAST_RESULT = {}

_SHAPES = {"x": ([C, HW], F32), "x16": ([C, HW], F16),
           "wconv": ([128, 1408], F16), "wqkv": ([128, 448], F16),
           "wbias": ([128, 9], F32)}


def _build_nc():
    if "nc" in _NC_CACHE:
        return _NC_CACHE["nc"]
    nc = bacc.Bacc(trn_type="TRN2", target_bir_lowering=False, debug=False)
    a = {}
    for n, (s, dt) in _SHAPES.items():
        a[n] = nc.dram_tensor(n, s, dt, kind="ExternalInput").ap()
    a["y"] = nc.dram_tensor("y", [C, HW], F32, kind="ExternalOutput").ap()
    if _kernel_body.__globals__["DEBUG"]:
        dbg = {"dbg_sc0": [128, 8192], "dbg_sc1": [128, 8192],
               "dbg_zqv0": [128, HW], "dbg_zqv1": [128, HW],
               "dbg_zk0": [128, 8192], "dbg_zk1": [128, 8192],
               "dbg_gsb": [128, 2048], "dbg_zs": [128, HW],
               "dbg_scp": [128, HW]}
        for n, s in dbg.items():
            a[n] = nc.dram_tensor(n, s, F16, kind="ExternalOutput").ap()
    with tile.TileContext(nc) as tc:
        _kernel_body(tc, a)
    nc.compile()
    _NC_CACHE["nc"] = nc
    return nc


def _in_maps(inputs):
    w = _prep_weights(inputs)
    x = np.ascontiguousarray(np.asarray(inputs["x"], dtype=np.float32))
    maps = []
    for core in range(N_CORES):
        xc = np.ascontiguousarray(x[core].reshape(C, HW))
        m = {"x": xc, "x16": xc.astype(np.float16)}
        m.update(w)
        maps.append(m)
    return maps


def kernel(**inputs):
    from concourse.bass_utils import run_bass_kernel_spmd

    nc = _build_nc()
    res = run_bass_kernel_spmd(nc, _in_maps(inputs), core_ids=list(range(N_CORES)),
                               trace=_RUN_OPTS["trace"])
    _LAST_RESULT["res"] = res
    out = np.stack([res.results[i]["y"].reshape(C, H, W) for i in range(N_CORES)])
    return out.astype(np.float32)


if __name__ == "__main__":
    nc = _build_nc()
    print("built ok")
